# revision 18
# baseline (speedup 1.0000x reference)
"""ComplexGRUCell forward on 8 Trainium2 NeuronCores.

Strategy (data-parallel, feat-major compute), v3:
  - Shard batch B=65536 across 8 cores (8192 rows each).
  - Gate pre-activations (r, z) via fp8-e4m3 DoubleRowSwInterleave
    matmuls (2x PE throughput, software-interleaved weights so the
    weight loads read contiguously). The moving data is split hi+lo
    (error feedback): two DR matmuls per weight chunk accumulate
    w*(x_hi) + w*(x_lo), cancelling the moving-side quantization error.
    Host scales moving data by 16 and weights by 256; the sigmoid
    un-scales via its scale argument.
  - Candidate pre-activations (x3, g3) via fp16 matmuls.
  - All element-wise epilogue work in fp16 SBUF (2x packed DVE mode).
  - Streams shipped feature-major, tile-major interleaved so each batch
    tile needs ONE dma per stream class (fp16 / fp8-hi / fp8-lo).
  - Outputs written fp16 [256, 2, 8192]; host transposes/upcasts.

Self-contained: hardcodes B=65536, I=H=256, 8 cores.
"""

import numpy as np
import ml_dtypes

import concourse.bass as bass  # noqa: F401
import concourse.mybir as mybir
import concourse.tile as tile
from concourse import bacc, bass_utils

F32 = mybir.dt.float32
F16 = mybir.dt.float16
F8 = mybir.dt.float8e4
AF = mybir.ActivationFunctionType
PM = mybir.MatmulPerfMode

B_TOTAL = 65536
N_CORES = 8
B_LOC = B_TOTAL // N_CORES  # 8192
H = 256
NB = 512                    # batch columns per tile
N_TILES = B_LOC // NB       # 16

S_MOV = 16.0                # fp8 moving-data scale
S_WGT = 256.0               # fp8 weight scale
INV_S = 1.0 / (S_MOV * S_WGT)
E4M3 = ml_dtypes.float8_e4m3

GATE_PERF = PM.DoubleRow

_STREAMS = ["xr", "xi", "hr", "hi"]
_R_ACCS = ["r_re", "r_im"]                       # fp8 DoubleRow
_Z_ACCS = ["z_re", "z_im"]                       # fp16 (precision-critical)
_GATE_ACCS = _R_ACCS + _Z_ACCS
_CAND_ACCS = ["x3_re", "x3_im", "g3_re", "g3_im"]

# Module-level knobs for the test harness (grading path leaves them alone).
TRACE = False
LAST_RESULT = None

_CACHED_NC = None


def _build_nc():
    nc = bacc.Bacc("TRN2", target_bir_lowering=False, debug=False,
                   num_devices=N_CORES)

    ins = {}
    # tile-major interleaved streams: [128, t, 2*si + c, b]
    ins["s16"] = nc.dram_tensor("s16", (128, N_TILES * 8 * NB), F16,
                                kind="ExternalInput")
    ins["s8h"] = nc.dram_tensor("s8h", (128, N_TILES * 8 * NB), F8,
                                kind="ExternalInput")
    # r-gate weights (fp8, DR pair layout): per (acc,mo): 4 si-blocks
    ins["w8"] = nc.dram_tensor("w8", (128, 4, 4 * 256), F8,
                               kind="ExternalInput")
    # z-gate weights fp16: per (acc,mo): 8 chunks of 128 cols
    ins["w16z"] = nc.dram_tensor("w16z", (128, 4, 8 * 128), F16,
                                 kind="ExternalInput")
    # cand weights fp16: per (acc,mo): 4 chunks of 128 cols
    ins["w16"] = nc.dram_tensor("w16", (128, 8, 4 * 128), F16,
                                kind="ExternalInput")
    ins["biases"] = nc.dram_tensor("biases", (128, 16), F32,
                                   kind="ExternalInput")
    # output: [feature, re/im, batch]
    outT = nc.dram_tensor("outT", (H, 2, B_LOC), F16, kind="ExternalOutput")

    r_idx = {g: i for i, g in enumerate(_R_ACCS)}
    z_idx = {g: i for i, g in enumerate(_Z_ACCS)}
    cand_idx = {g: i for i, g in enumerate(_CAND_ACCS)}
    bias_col = {}
    for gi, g in enumerate(_GATE_ACCS + _CAND_ACCS):
        for mo in range(2):
            bias_col[(g, mo)] = gi * 2 + mo

    with tile.TileContext(nc) as tc:
        with (
            tc.tile_pool(name="wpool", bufs=1) as wpool,
            tc.tile_pool(name="m8pool", bufs=2) as m8pool,
            tc.tile_pool(name="m16pool", bufs=3) as m16pool,
            tc.tile_pool(name="spool", bufs=2) as spool,
            tc.tile_pool(name="tpool", bufs=2) as tpool,
            tc.tile_pool(name="opool", bufs=3) as opool,
            tc.tile_pool(name="psum", bufs=1, space="PSUM") as psum,
        ):
            # ---- one-time weight/bias loads -------------------------------
            # [128, (row*4+si)*2 + j, 128]: DR pair dim must be its own axis
            w8t = wpool.tile([128, 32, 128], F8, name="w8t", tag="w8t")
            nc.sync.dma_start(w8t[:], ins["w8"][:])

            def load_m8(c0):
                t0 = c0 // NB * (8 * NB)
                h8 = m8pool.tile([128, 8, NB], F8, name="m8h", tag="m8h")
                nc.sync.dma_start(h8[:], ins["s8h"][:, t0:t0 + 8 * NB])
                return h8

            def load_m16(c0):
                t0 = c0 // NB * (8 * NB)
                t = m16pool.tile([128, 8, NB], F16, name="m16", tag="m16")
                nc.sync.dma_start(t[:], ins["s16"][:, t0:t0 + 8 * NB])
                return t

            m8_0 = load_m8(0)
            wzt = wpool.tile([128, 4, 8 * 128], F16, name="wzt", tag="wzt")
            nc.sync.dma_start(wzt[:], ins["w16z"][:])
            m16_0 = load_m16(0)
            w16t = wpool.tile([128, 8, 4 * 128], F16, name="w16t", tag="w16t")
            nc.sync.dma_start(w16t[:], ins["w16"][:])
            bt = wpool.tile([128, 16], F32, name="bias_t", tag="bias_t")
            nc.sync.dma_start(bt[:], ins["biases"][:])

            def bias_ap(g, mo):
                c = bias_col[(g, mo)]
                return bt[:, c:c + 1]

            # ---- per batch tile -------------------------------------------
            for t_idx in range(N_TILES):
                c0 = t_idx * NB
                if t_idx == 0:
                    m8h, m16 = m8_0, m16_0
                else:
                    m8h = load_m8(c0)
                    m16 = load_m16(c0)

                for mo in range(2):
                    p_r = psum.tile([128, 2 * NB], F32, name=f"pr{mo}",
                                    tag="bkA")
                    p_z = psum.tile([128, 2 * NB], F32, name=f"pz{mo}",
                                    tag="bkB")

                    def r_mm(dst, g, mo, si):
                        k = ((r_idx[g] * 2 + mo) * 4 + si) * 2
                        nc.tensor.matmul(
                            dst, w8t[:, k:k + 2, :],
                            m8h[:, 2 * si:2 * si + 2, :],
                            start=(si == 0), stop=(si == 3),
                            perf_mode=GATE_PERF)

                    def z_mm(dst, g, mo, j):
                        wrow = z_idx[g] * 2 + mo
                        nc.tensor.matmul(
                            dst, wzt[:, wrow, j * 128:(j + 1) * 128],
                            m16[:, j, :], start=(j == 0), stop=(j == 7))

                    # Interleave fp8-DR r matmuls between fp16 z matmuls so
                    # the DR weight loads overlap the longer fp16 streams.
                    z_seq = [(p_z[:, 0:NB], "z_re", j) for j in range(8)] + \
                            [(p_z[:, NB:], "z_im", j) for j in range(8)]
                    r_seq = [(p_r[:, 0:NB], "r_re", si) for si in range(4)] + \
                            [(p_r[:, NB:], "r_im", si) for si in range(4)]
                    for i in range(8):
                        dst, g, j = z_seq[2 * i]
                        z_mm(dst, g, mo, j)
                        dst, g, j = z_seq[2 * i + 1]
                        z_mm(dst, g, mo, j)
                        dst, g, si = r_seq[i]
                        r_mm(dst, g, mo, si)

                    p_x3 = psum.tile([128, 2 * NB], F32, name=f"px{mo}",
                                     tag="bkC")
                    p_g3 = psum.tile([128, 2 * NB], F32, name=f"pg{mo}",
                                     tag="bkD")

                    def cand_accum(dst, g, mo, j0):
                        wrow = cand_idx[g] * 2 + mo
                        for k in range(4):
                            nc.tensor.matmul(
                                dst, w16t[:, wrow, k * 128:(k + 1) * 128],
                                m16[:, j0 + k, :], start=(k == 0),
                                stop=(k == 3))

                    cand_accum(p_x3[:, 0:NB], "x3_re", mo, 0)
                    cand_accum(p_x3[:, NB:], "x3_im", mo, 0)
                    cand_accum(p_g3[:, 0:NB], "g3_re", mo, 4)
                    cand_accum(p_g3[:, NB:], "g3_im", mo, 4)

                    # ---- elementwise epilogue ------------------------------
                    sr = spool.tile([128, 2 * NB], F16, name=f"sr{mo}",
                                    tag="sr")
                    sz = spool.tile([128, 2 * NB], F16, name=f"sz{mo}",
                                    tag="sz")
                    g3 = spool.tile([128, 2 * NB], F16, name=f"g3{mo}",
                                    tag="g3")
                    nc.scalar.activation(sr[:, 0:NB], p_r[:, 0:NB],
                                         AF.Sigmoid, bias=bias_ap("r_re", mo),
                                         scale=INV_S)
                    nc.scalar.activation(sr[:, NB:], p_r[:, NB:],
                                         AF.Sigmoid, bias=bias_ap("r_im", mo),
                                         scale=INV_S)
                    nc.scalar.activation(sz[:, 0:NB], p_z[:, 0:NB],
                                         AF.Sigmoid, bias=bias_ap("z_re", mo))
                    nc.scalar.activation(sz[:, NB:], p_z[:, NB:],
                                         AF.Sigmoid, bias=bias_ap("z_im", mo))
                    nc.scalar.activation(g3[:, 0:NB], p_g3[:, 0:NB],
                                         AF.Identity,
                                         bias=bias_ap("g3_re", mo))
                    nc.scalar.activation(g3[:, NB:], p_g3[:, NB:],
                                         AF.Identity,
                                         bias=bias_ap("g3_im", mo))

                    # h3 = r * g3 (complex), all fp16 SBUF (2x DVE mode)
                    u = tpool.tile([128, 2 * NB], F16, name=f"u{mo}", tag="u")
                    v = tpool.tile([128, 2 * NB], F16, name=f"v{mo}", tag="v")
                    h3 = tpool.tile([128, 2 * NB], F16, name=f"h3{mo}",
                                    tag="h3")
                    nc.vector.tensor_mul(u[:], sr[:], g3[:])
                    nc.vector.tensor_mul(v[:, 0:NB], sr[:, 0:NB], g3[:, NB:])
                    nc.vector.tensor_mul(v[:, NB:], sr[:, NB:], g3[:, 0:NB])
                    nc.vector.tensor_sub(h3[:, 0:NB], u[:, 0:NB], u[:, NB:])
                    nc.vector.tensor_add(h3[:, NB:], v[:, 0:NB], v[:, NB:])
                    # ss = x3 + h3 (PSUM read, 1x); tanh adds x3 bias
                    ss = tpool.tile([128, 2 * NB], F16, name=f"ss{mo}",
                                    tag="ss")
                    nc.vector.tensor_add(ss[:], p_x3[:], h3[:])
                    nn = spool.tile([128, 2 * NB], F16, name=f"nn{mo}",
                                    tag="nn")
                    nc.scalar.activation(nn[:, 0:NB], ss[:, 0:NB], AF.Tanh,
                                         bias=bias_ap("x3_re", mo))
                    nc.scalar.activation(nn[:, NB:], ss[:, NB:], AF.Tanh,
                                         bias=bias_ap("x3_im", mo))

                    # d = h - n ; out = n + z*d (complex)
                    d = tpool.tile([128, 2 * NB], F16, name=f"d{mo}", tag="d")
                    p = tpool.tile([128, 2 * NB], F16, name=f"p{mo}", tag="p")
                    q = tpool.tile([128, 2 * NB], F16, name=f"q{mo}", tag="q")
                    tm = tpool.tile([128, 2 * NB], F16, name=f"tm{mo}",
                                    tag="tm")
                    ot = opool.tile([128, 2, NB], F16, name=f"ot{mo}",
                                    tag="ot")
                    nc.vector.tensor_sub(d[:, 0:NB], m16[:, 4 + mo, :],
                                         nn[:, 0:NB])
                    nc.vector.tensor_sub(d[:, NB:], m16[:, 6 + mo, :],
                                         nn[:, NB:])
                    nc.vector.tensor_mul(p[:], sz[:], d[:])
                    nc.vector.tensor_mul(q[:, 0:NB], sz[:, 0:NB], d[:, NB:])
                    nc.vector.tensor_mul(q[:, NB:], sz[:, NB:], d[:, 0:NB])
                    nc.vector.tensor_sub(tm[:, 0:NB], p[:, 0:NB], p[:, NB:])
                    nc.vector.tensor_add(tm[:, NB:], q[:, 0:NB], q[:, NB:])
                    nc.vector.tensor_add(ot[:, 0, :], nn[:, 0:NB],
                                         tm[:, 0:NB])
                    nc.vector.tensor_add(ot[:, 1, :], nn[:, NB:], tm[:, NB:])

                    # one DMA per mo: [128 feat, 2 (re/im), NB]
                    nc.sync.dma_start(
                        outT[mo * 128:(mo + 1) * 128, :, c0:c0 + NB], ot[:])

    nc.compile()
    return nc


def _stack_stat(p, g):
    """Stationary matrix [K, 256] for accumulator g (K = 1024 or 512)."""
    blocks = {
        "r_re": [p["w1Wr"], -p["w1Wi"], p["r1Wr"], -p["r1Wi"]],
        "r_im": [p["w1Wi"], p["w1Wr"], p["r1Wi"], p["r1Wr"]],
        "z_re": [p["w2Wr"], -p["w2Wi"], p["r2Wr"], -p["r2Wi"]],
        "z_im": [p["w2Wi"], p["w2Wr"], p["r2Wi"], p["r2Wr"]],
        "x3_re": [p["w3Wr"], -p["w3Wi"]],
        "x3_im": [p["w3Wi"], p["w3Wr"]],
        "g3_re": [p["r3Wr"], -p["r3Wi"]],
        "g3_im": [p["r3Wi"], p["r3Wr"]],
    }[g]
    return np.concatenate([np.asarray(W, np.float32).T for W in blocks],
                          axis=0)


def _pack_gate_pair(w0, w1):
    """Pack a chunk pair [128,128]x2 into the DR weight layout [128, 256]."""
    if GATE_PERF == PM.DoubleRowSwInterleave:
        # flat[p, 2*(127-m) + i] = w_i[p, m]
        arr = np.stack([w0[:, ::-1], w1[:, ::-1]], axis=2)  # [p, m', i]
        return arr.reshape(128, 256)
    # plain DoubleRow: [p, i, m]
    return np.stack([w0, w1], axis=1).reshape(128, 256)


def _prep_weights(p):
    w8 = np.zeros((128, 4, 4 * 256), dtype=np.float32)
    for g in _R_ACCS:
        stat = _stack_stat(p, g)  # [1024, 256]
        for mo in range(2):
            sub = stat[:, mo * 128:(mo + 1) * 128] * S_WGT  # [1024, 128]
            gi = _R_ACCS.index(g)
            for si in range(4):
                w0 = sub[si * 256:si * 256 + 128]
                w1 = sub[si * 256 + 128:(si + 1) * 256]
                w8[:, gi * 2 + mo, si * 256:(si + 1) * 256] = \
                    _pack_gate_pair(w0, w1)
    w16z = np.zeros((128, 4, 8 * 128), dtype=np.float16)
    for g in _Z_ACCS:
        stat = _stack_stat(p, g)  # [1024, 256]
        for mo in range(2):
            sub = stat[:, mo * 128:(mo + 1) * 128]
            gi = _Z_ACCS.index(g)
            for k in range(8):
                w16z[:, gi * 2 + mo, k * 128:(k + 1) * 128] = \
                    sub[k * 128:(k + 1) * 128].astype(np.float16)
    w16 = np.zeros((128, 8, 4 * 128), dtype=np.float16)
    for g in _CAND_ACCS:
        stat = _stack_stat(p, g)  # [512, 256]
        for mo in range(2):
            sub = stat[:, mo * 128:(mo + 1) * 128]
            gi = _CAND_ACCS.index(g)
            for k in range(4):
                w16[:, gi * 2 + mo, k * 128:(k + 1) * 128] = \
                    sub[k * 128:(k + 1) * 128].astype(np.float16)

    bias = {
        "r_re": p["w1br"] - p["w1bi"] + p["r1br"] - p["r1bi"],
        "r_im": p["w1br"] + p["w1bi"] + p["r1br"] + p["r1bi"],
        "z_re": p["w2br"] - p["w2bi"] + p["r2br"] - p["r2bi"],
        "z_im": p["w2br"] + p["w2bi"] + p["r2br"] + p["r2bi"],
        "x3_re": p["w3br"] - p["w3bi"],
        "x3_im": p["w3br"] + p["w3bi"],
        "g3_re": p["r3br"] - p["r3bi"],
        "g3_im": p["r3br"] + p["r3bi"],
    }
    bcols = np.zeros((128, 16), dtype=np.float32)
    for gi, g in enumerate(_GATE_ACCS + _CAND_ACCS):
        for mo in range(2):
            bcols[:, gi * 2 + mo] = np.asarray(bias[g])[mo * 128:(mo + 1) * 128]
    return {
        "w8": np.clip(w8, -240.0, 240.0).astype(E4M3),
        "w16z": w16z,
        "w16": w16,
        "biases": bcols,
    }


def kernel(**inputs):
    global _CACHED_NC, LAST_RESULT
    if _CACHED_NC is None:
        _CACHED_NC = _build_nc()
    nc = _CACHED_NC

    wmaps = _prep_weights(inputs)

    in_maps = []
    for core in range(N_CORES):
        sl = slice(core * B_LOC, (core + 1) * B_LOC)
        m = dict(wmaps)
        # [4 streams][256 feat, 8192 batch]
        xT = np.stack([
            np.asarray(inputs[s], np.float32)[sl].T for s in _STREAMS])
        # -> [128, t, 2*si + c, b] : chunks c over feature dim
        xTi = (xT.reshape(4, 2, 128, N_TILES, NB)
               .transpose(2, 3, 0, 1, 4))          # [128, t, si, c, b]
        m["s16"] = np.ascontiguousarray(
            xTi.reshape(128, -1).astype(np.float16))
        sc = np.clip(xTi * S_MOV, -240.0, 240.0)
        m["s8h"] = np.ascontiguousarray(sc.astype(E4M3).reshape(128, -1))
        in_maps.append(m)

    kwargs = {}
    if TRACE:
        import sys, types
        try:
            from trn_agent_boot.trn_boot import _ntff_profile_via_ctypes
            mod = types.ModuleType("antenv.axon_hooks")
            mod._hook = _ntff_profile_via_ctypes('/opt/axon/libaxon_pjrt.so')
            mod.get_axon_ntff_profile_hook = lambda: mod._hook
            mod.set_axon_ntff_profile_hook = (
                lambda h: setattr(mod, "_hook", h))
            sys.modules["antenv.axon_hooks"] = mod
            kwargs["trace"] = True
        except Exception:
            pass

    res = bass_utils.run_bass_kernel_spmd(
        nc, in_maps, core_ids=list(range(N_CORES)), **kwargs)
    LAST_RESULT = res

    out = np.empty((2, B_TOTAL, H), dtype=np.float32)
    for core in range(N_CORES):
        sl = slice(core * B_LOC, (core + 1) * B_LOC)
        o = np.asarray(res.results[core]["outT"], dtype=np.float32)
        out[0, sl] = o[:, 0, :].T
        out[1, sl] = o[:, 1, :].T
    return out


# revision 20
# speedup vs baseline: 1.3173x; 1.3173x over previous
"""ComplexGRUCell forward on 8 Trainium2 NeuronCores.

Strategy (data-parallel, feat-major compute), v3:
  - Shard batch B=65536 across 8 cores (8192 rows each).
  - Gate pre-activations (r, z) via fp8-e4m3 DoubleRowSwInterleave
    matmuls (2x PE throughput, software-interleaved weights so the
    weight loads read contiguously). The moving data is split hi+lo
    (error feedback): two DR matmuls per weight chunk accumulate
    w*(x_hi) + w*(x_lo), cancelling the moving-side quantization error.
    Host scales moving data by 16 and weights by 256; the sigmoid
    un-scales via its scale argument.
  - Candidate pre-activations (x3, g3) via fp16 matmuls.
  - All element-wise epilogue work in fp16 SBUF (2x packed DVE mode).
  - Streams shipped feature-major, tile-major interleaved so each batch
    tile needs ONE dma per stream class (fp16 / fp8-hi / fp8-lo).
  - Outputs written fp16 [256, 2, 8192]; host transposes/upcasts.

Self-contained: hardcodes B=65536, I=H=256, 8 cores.
"""

import numpy as np
import ml_dtypes

import concourse.bass as bass  # noqa: F401
import concourse.mybir as mybir
import concourse.tile as tile
from concourse import bacc, bass_utils

F32 = mybir.dt.float32
F16 = mybir.dt.float16
F8 = mybir.dt.float8e4
AF = mybir.ActivationFunctionType
PM = mybir.MatmulPerfMode

B_TOTAL = 65536
N_CORES = 8
B_LOC = B_TOTAL // N_CORES  # 8192
H = 256
NB = 512                    # batch columns per tile
N_TILES = B_LOC // NB       # 16

S_MOV = 16.0                # fp8 moving-data scale
S_WGT = 256.0               # fp8 weight scale
INV_S = 1.0 / (S_MOV * S_WGT)
E4M3 = ml_dtypes.float8_e4m3

GATE_PERF = PM.DoubleRow

_STREAMS = ["xr", "xi", "hr", "hi"]
_R_ACCS = ["r_re", "r_im"]                       # fp8 DoubleRow
_Z_ACCS = ["z_re", "z_im"]                       # fp16 (precision-critical)
_GATE_ACCS = _R_ACCS + _Z_ACCS
_CAND_ACCS = ["x3_re", "x3_im", "g3_re", "g3_im"]

# Module-level knobs for the test harness (grading path leaves them alone).
TRACE = False
LAST_RESULT = None

_CACHED_NC = None


def _build_nc():
    nc = bacc.Bacc("TRN2", target_bir_lowering=False, debug=False,
                   num_devices=N_CORES)

    ins = {}
    # tile-major interleaved streams: [128, t, 2*si + c, b]
    ins["s16"] = nc.dram_tensor("s16", (128, N_TILES * 8 * NB), F16,
                                kind="ExternalInput")
    ins["s8h"] = nc.dram_tensor("s8h", (128, N_TILES * 8 * NB), F8,
                                kind="ExternalInput")
    # r-gate weights (fp8, DR pair layout): per (acc,mo): 4 si-blocks
    ins["w8"] = nc.dram_tensor("w8", (128, 48, 128), F8,
                               kind="ExternalInput")
    # z-gate h-part weights fp16 (x4096): per (acc,mo): 4 chunks of 128
    ins["w16z"] = nc.dram_tensor("w16z", (128, 4, 4 * 128), F16,
                                 kind="ExternalInput")
    # cand weights fp16: per (acc,mo): 4 chunks of 128 cols
    ins["w16"] = nc.dram_tensor("w16", (128, 8, 4 * 128), F16,
                                kind="ExternalInput")
    ins["biases"] = nc.dram_tensor("biases", (128, 16), F32,
                                   kind="ExternalInput")
    # output: [feature, re/im, batch]
    outT = nc.dram_tensor("outT", (H, 2, B_LOC), F16, kind="ExternalOutput")

    r_idx = {g: i for i, g in enumerate(_R_ACCS)}
    z_idx = {g: i for i, g in enumerate(_Z_ACCS)}
    cand_idx = {g: i for i, g in enumerate(_CAND_ACCS)}
    bias_col = {}
    for gi, g in enumerate(_GATE_ACCS + _CAND_ACCS):
        for mo in range(2):
            bias_col[(g, mo)] = gi * 2 + mo

    with tile.TileContext(nc) as tc:
        with (
            tc.tile_pool(name="wpool", bufs=1) as wpool,
            tc.tile_pool(name="m8pool", bufs=2) as m8pool,
            tc.tile_pool(name="m16pool", bufs=3) as m16pool,
            tc.tile_pool(name="spool", bufs=2) as spool,
            tc.tile_pool(name="tpool", bufs=2) as tpool,
            tc.tile_pool(name="opool", bufs=3) as opool,
            tc.tile_pool(name="psum", bufs=1, space="PSUM") as psum,
        ):
            # ---- one-time weight/bias loads -------------------------------
            # [128, (row*4+si)*2 + j, 128]: DR pair dim must be its own axis
            w8t = wpool.tile([128, 48, 128], F8, name="w8t", tag="w8t")
            nc.sync.dma_start(w8t[:], ins["w8"][:])

            def load_m8(c0):
                t0 = c0 // NB * (8 * NB)
                h8 = m8pool.tile([128, 8, NB], F8, name="m8h", tag="m8h")
                nc.sync.dma_start(h8[:], ins["s8h"][:, t0:t0 + 8 * NB])
                return h8

            def load_m16(c0):
                t0 = c0 // NB * (8 * NB)
                t = m16pool.tile([128, 8, NB], F16, name="m16", tag="m16")
                nc.sync.dma_start(t[:], ins["s16"][:, t0:t0 + 8 * NB])
                return t

            m8_0 = load_m8(0)
            wzt = wpool.tile([128, 4, 4 * 128], F16, name="wzt", tag="wzt")
            nc.sync.dma_start(wzt[:], ins["w16z"][:])
            m16_0 = load_m16(0)
            w16t = wpool.tile([128, 8, 4 * 128], F16, name="w16t", tag="w16t")
            nc.sync.dma_start(w16t[:], ins["w16"][:])
            bt = wpool.tile([128, 16], F32, name="bias_t", tag="bias_t")
            nc.sync.dma_start(bt[:], ins["biases"][:])

            def bias_ap(g, mo):
                c = bias_col[(g, mo)]
                return bt[:, c:c + 1]

            # ---- per batch tile -------------------------------------------
            for t_idx in range(N_TILES):
                c0 = t_idx * NB
                if t_idx == 0:
                    m8h, m16 = m8_0, m16_0
                else:
                    m8h = load_m8(c0)
                    m16 = load_m16(c0)

                for mo in range(2):
                    p_r = psum.tile([128, 2 * NB], F32, name=f"pr{mo}",
                                    tag="bkA")
                    p_z = psum.tile([128, 2 * NB], F32, name=f"pz{mo}",
                                    tag="bkB")

                    def r_accum(dst, g, mo):
                        # 4 DR matmuls over all-stream chunk pairs
                        for si in range(4):
                            k = ((r_idx[g] * 2 + mo) * 4 + si) * 2
                            nc.tensor.matmul(
                                dst, w8t[:, k:k + 2, :],
                                m8h[:, 2 * si:2 * si + 2, :],
                                start=(si == 0), stop=(si == 3),
                                perf_mode=GATE_PERF)

                    def zx_accum(dst, g, mo):
                        # x-part of z in fp8 DR: 2 DR matmuls (si = 0, 1)
                        for si in range(2):
                            k = (16 + (z_idx[g] * 2 + mo) * 2 + si) * 2
                            nc.tensor.matmul(
                                dst, w8t[:, k:k + 2, :],
                                m8h[:, 2 * si:2 * si + 2, :],
                                start=(si == 0), stop=False,
                                perf_mode=GATE_PERF)

                    def zh_accum(dst, g, mo):
                        # h-part of z in fp16 (weights pre-scaled by 4096)
                        wrow = z_idx[g] * 2 + mo
                        for j in range(4):
                            nc.tensor.matmul(
                                dst, wzt[:, wrow, j * 128:(j + 1) * 128],
                                m16[:, 4 + j, :], start=False, stop=(j == 3))

                    # group DR matmuls together (mode switches are costly)
                    r_accum(p_r[:, 0:NB], "r_re", mo)
                    r_accum(p_r[:, NB:], "r_im", mo)
                    zx_accum(p_z[:, 0:NB], "z_re", mo)
                    zx_accum(p_z[:, NB:], "z_im", mo)
                    zh_accum(p_z[:, 0:NB], "z_re", mo)
                    zh_accum(p_z[:, NB:], "z_im", mo)

                    p_x3 = psum.tile([128, 2 * NB], F32, name=f"px{mo}",
                                     tag="bkC")
                    p_g3 = psum.tile([128, 2 * NB], F32, name=f"pg{mo}",
                                     tag="bkD")

                    def cand_accum(dst, g, mo, j0):
                        wrow = cand_idx[g] * 2 + mo
                        for k in range(4):
                            nc.tensor.matmul(
                                dst, w16t[:, wrow, k * 128:(k + 1) * 128],
                                m16[:, j0 + k, :], start=(k == 0),
                                stop=(k == 3))

                    cand_accum(p_x3[:, 0:NB], "x3_re", mo, 0)
                    cand_accum(p_x3[:, NB:], "x3_im", mo, 0)
                    cand_accum(p_g3[:, 0:NB], "g3_re", mo, 4)
                    cand_accum(p_g3[:, NB:], "g3_im", mo, 4)

                    # ---- elementwise epilogue ------------------------------
                    sr = spool.tile([128, 2 * NB], F16, name=f"sr{mo}",
                                    tag="sr")
                    sz = spool.tile([128, 2 * NB], F16, name=f"sz{mo}",
                                    tag="sz")
                    g3 = spool.tile([128, 2 * NB], F16, name=f"g3{mo}",
                                    tag="g3")
                    nc.scalar.activation(sr[:, 0:NB], p_r[:, 0:NB],
                                         AF.Sigmoid, bias=bias_ap("r_re", mo),
                                         scale=INV_S)
                    nc.scalar.activation(sr[:, NB:], p_r[:, NB:],
                                         AF.Sigmoid, bias=bias_ap("r_im", mo),
                                         scale=INV_S)
                    nc.scalar.activation(sz[:, 0:NB], p_z[:, 0:NB],
                                         AF.Sigmoid, bias=bias_ap("z_re", mo),
                                         scale=INV_S)
                    nc.scalar.activation(sz[:, NB:], p_z[:, NB:],
                                         AF.Sigmoid, bias=bias_ap("z_im", mo),
                                         scale=INV_S)
                    nc.scalar.activation(g3[:, 0:NB], p_g3[:, 0:NB],
                                         AF.Identity,
                                         bias=bias_ap("g3_re", mo))
                    nc.scalar.activation(g3[:, NB:], p_g3[:, NB:],
                                         AF.Identity,
                                         bias=bias_ap("g3_im", mo))

                    # h3 = r * g3 (complex), all fp16 SBUF (2x DVE mode)
                    u = tpool.tile([128, 2 * NB], F16, name=f"u{mo}", tag="u")
                    v = tpool.tile([128, 2 * NB], F16, name=f"v{mo}", tag="v")
                    h3 = tpool.tile([128, 2 * NB], F16, name=f"h3{mo}",
                                    tag="h3")
                    nc.vector.tensor_mul(u[:], sr[:], g3[:])
                    nc.vector.tensor_mul(v[:, 0:NB], sr[:, 0:NB], g3[:, NB:])
                    nc.vector.tensor_mul(v[:, NB:], sr[:, NB:], g3[:, 0:NB])
                    nc.vector.tensor_sub(h3[:, 0:NB], u[:, 0:NB], u[:, NB:])
                    nc.vector.tensor_add(h3[:, NB:], v[:, 0:NB], v[:, NB:])
                    # ss = x3 + h3 (PSUM read, 1x); tanh adds x3 bias
                    ss = tpool.tile([128, 2 * NB], F16, name=f"ss{mo}",
                                    tag="ss")
                    nc.vector.tensor_add(ss[:], p_x3[:], h3[:])
                    nn = spool.tile([128, 2 * NB], F16, name=f"nn{mo}",
                                    tag="nn")
                    nc.scalar.activation(nn[:, 0:NB], ss[:, 0:NB], AF.Tanh,
                                         bias=bias_ap("x3_re", mo))
                    nc.scalar.activation(nn[:, NB:], ss[:, NB:], AF.Tanh,
                                         bias=bias_ap("x3_im", mo))

                    # d = h - n ; out = n + z*d (complex)
                    d = tpool.tile([128, 2 * NB], F16, name=f"d{mo}", tag="d")
                    p = tpool.tile([128, 2 * NB], F16, name=f"p{mo}", tag="p")
                    q = tpool.tile([128, 2 * NB], F16, name=f"q{mo}", tag="q")
                    tm = tpool.tile([128, 2 * NB], F16, name=f"tm{mo}",
                                    tag="tm")
                    ot = opool.tile([128, 2, NB], F16, name=f"ot{mo}",
                                    tag="ot")
                    nc.vector.tensor_sub(d[:, 0:NB], m16[:, 4 + mo, :],
                                         nn[:, 0:NB])
                    nc.vector.tensor_sub(d[:, NB:], m16[:, 6 + mo, :],
                                         nn[:, NB:])
                    nc.vector.tensor_mul(p[:], sz[:], d[:])
                    nc.vector.tensor_mul(q[:, 0:NB], sz[:, 0:NB], d[:, NB:])
                    nc.vector.tensor_mul(q[:, NB:], sz[:, NB:], d[:, 0:NB])
                    nc.vector.tensor_sub(tm[:, 0:NB], p[:, 0:NB], p[:, NB:])
                    nc.vector.tensor_add(tm[:, NB:], q[:, 0:NB], q[:, NB:])
                    nc.vector.tensor_add(ot[:, 0, :], nn[:, 0:NB],
                                         tm[:, 0:NB])
                    nc.vector.tensor_add(ot[:, 1, :], nn[:, NB:], tm[:, NB:])

                    # one DMA per mo: [128 feat, 2 (re/im), NB]
                    nc.sync.dma_start(
                        outT[mo * 128:(mo + 1) * 128, :, c0:c0 + NB], ot[:])

    nc.compile()
    return nc


def _stack_stat(p, g):
    """Stationary matrix [K, 256] for accumulator g (K = 1024 or 512)."""
    blocks = {
        "r_re": [p["w1Wr"], -p["w1Wi"], p["r1Wr"], -p["r1Wi"]],
        "r_im": [p["w1Wi"], p["w1Wr"], p["r1Wi"], p["r1Wr"]],
        "z_re": [p["w2Wr"], -p["w2Wi"], p["r2Wr"], -p["r2Wi"]],
        "z_im": [p["w2Wi"], p["w2Wr"], p["r2Wi"], p["r2Wr"]],
        "x3_re": [p["w3Wr"], -p["w3Wi"]],
        "x3_im": [p["w3Wi"], p["w3Wr"]],
        "g3_re": [p["r3Wr"], -p["r3Wi"]],
        "g3_im": [p["r3Wi"], p["r3Wr"]],
    }[g]
    return np.concatenate([np.asarray(W, np.float32).T for W in blocks],
                          axis=0)


def _pack_gate_pair(w0, w1):
    """Pack a chunk pair [128,128]x2 into the DR weight layout [128, 256]."""
    if GATE_PERF == PM.DoubleRowSwInterleave:
        # flat[p, 2*(127-m) + i] = w_i[p, m]
        arr = np.stack([w0[:, ::-1], w1[:, ::-1]], axis=2)  # [p, m', i]
        return arr.reshape(128, 256)
    # plain DoubleRow: [p, i, m]
    return np.stack([w0, w1], axis=1).reshape(128, 256)


def _prep_weights(p):
    w8 = np.zeros((128, 48, 128), dtype=np.float32)
    for g in _R_ACCS:
        stat = _stack_stat(p, g)  # [1024, 256]
        for mo in range(2):
            sub = stat[:, mo * 128:(mo + 1) * 128] * S_WGT  # [1024, 128]
            gi = _R_ACCS.index(g)
            for si in range(4):
                k = ((gi * 2 + mo) * 4 + si) * 2
                w8[:, k] = sub[si * 256:si * 256 + 128]
                w8[:, k + 1] = sub[si * 256 + 128:(si + 1) * 256]
    w16z = np.zeros((128, 4, 4 * 128), dtype=np.float16)
    for g in _Z_ACCS:
        stat = _stack_stat(p, g)  # [1024, 256]
        gi = _Z_ACCS.index(g)
        for mo in range(2):
            sub = stat[:, mo * 128:(mo + 1) * 128] * S_WGT  # [1024, 128]
            # x-part (rows 0:512) -> fp8 DR pairs 16..23
            for si in range(2):
                k = (16 + (gi * 2 + mo) * 2 + si) * 2
                w8[:, k] = sub[si * 256:si * 256 + 128]
                w8[:, k + 1] = sub[si * 256 + 128:(si + 1) * 256]
            # h-part (rows 512:1024) -> fp16, scaled to match fp8 psum scale
            subh = sub[512:] * S_MOV
            for k in range(4):
                w16z[:, gi * 2 + mo, k * 128:(k + 1) * 128] = \
                    subh[k * 128:(k + 1) * 128].astype(np.float16)
    w16 = np.zeros((128, 8, 4 * 128), dtype=np.float16)
    for g in _CAND_ACCS:
        stat = _stack_stat(p, g)  # [512, 256]
        for mo in range(2):
            sub = stat[:, mo * 128:(mo + 1) * 128]
            gi = _CAND_ACCS.index(g)
            for k in range(4):
                w16[:, gi * 2 + mo, k * 128:(k + 1) * 128] = \
                    sub[k * 128:(k + 1) * 128].astype(np.float16)

    bias = {
        "r_re": p["w1br"] - p["w1bi"] + p["r1br"] - p["r1bi"],
        "r_im": p["w1br"] + p["w1bi"] + p["r1br"] + p["r1bi"],
        "z_re": p["w2br"] - p["w2bi"] + p["r2br"] - p["r2bi"],
        "z_im": p["w2br"] + p["w2bi"] + p["r2br"] + p["r2bi"],
        "x3_re": p["w3br"] - p["w3bi"],
        "x3_im": p["w3br"] + p["w3bi"],
        "g3_re": p["r3br"] - p["r3bi"],
        "g3_im": p["r3br"] + p["r3bi"],
    }
    bcols = np.zeros((128, 16), dtype=np.float32)
    for gi, g in enumerate(_GATE_ACCS + _CAND_ACCS):
        for mo in range(2):
            bcols[:, gi * 2 + mo] = np.asarray(bias[g])[mo * 128:(mo + 1) * 128]
    return {
        "w8": np.clip(w8, -240.0, 240.0).astype(E4M3),
        "w16z": w16z,
        "w16": w16,
        "biases": bcols,
    }


def kernel(**inputs):
    global _CACHED_NC, LAST_RESULT
    if _CACHED_NC is None:
        _CACHED_NC = _build_nc()
    nc = _CACHED_NC

    wmaps = _prep_weights(inputs)

    in_maps = []
    for core in range(N_CORES):
        sl = slice(core * B_LOC, (core + 1) * B_LOC)
        m = dict(wmaps)
        # [4 streams][256 feat, 8192 batch]
        xT = np.stack([
            np.asarray(inputs[s], np.float32)[sl].T for s in _STREAMS])
        # -> [128, t, 2*si + c, b] : chunks c over feature dim
        xTi = (xT.reshape(4, 2, 128, N_TILES, NB)
               .transpose(2, 3, 0, 1, 4))          # [128, t, si, c, b]
        m["s16"] = np.ascontiguousarray(
            xTi.reshape(128, -1).astype(np.float16))
        sc = np.clip(xTi * S_MOV, -240.0, 240.0)
        m["s8h"] = np.ascontiguousarray(sc.astype(E4M3).reshape(128, -1))
        in_maps.append(m)

    kwargs = {}
    if TRACE:
        import sys, types
        try:
            from trn_agent_boot.trn_boot import _ntff_profile_via_ctypes
            mod = types.ModuleType("antenv.axon_hooks")
            mod._hook = _ntff_profile_via_ctypes('/opt/axon/libaxon_pjrt.so')
            mod.get_axon_ntff_profile_hook = lambda: mod._hook
            mod.set_axon_ntff_profile_hook = (
                lambda h: setattr(mod, "_hook", h))
            sys.modules["antenv.axon_hooks"] = mod
            kwargs["trace"] = True
        except Exception:
            pass

    res = bass_utils.run_bass_kernel_spmd(
        nc, in_maps, core_ids=list(range(N_CORES)), **kwargs)
    LAST_RESULT = res

    out = np.empty((2, B_TOTAL, H), dtype=np.float32)
    for core in range(N_CORES):
        sl = slice(core * B_LOC, (core + 1) * B_LOC)
        o = np.asarray(res.results[core]["outT"], dtype=np.float32)
        out[0, sl] = o[:, 0, :].T
        out[1, sl] = o[:, 1, :].T
    return out
